# revision 1
# baseline (speedup 1.0000x reference)
"""BivectorRotarySelfAttention TRN2 kernel.

Sharding: 8 cores = 4 batches x 2 head-halves. Each core computes one batch's
attention for 8 heads (2 kv heads) and a partial output projection; host sums
the two head-half partials per batch.

Per-core dataflow (transposed layouts: features in partitions, seq in free):
  xT    = recombine(dma_transpose(x_hi), dma_transpose(x_lo))      [f32r]
  qT/kT/vT = W-blocks.T @ xT   (PSUM-accumulated f32r matmuls)
  rope via PE permutation-matmul + 2 DVE muls + 1 add
  scores S^T[m,q]: 4 K=64 matmuls (S0,S1 / C0,C1 row-packed pairs)
  raw = S0*S1 + c'*C0*C1 ; E = exp(alpha*raw + key_mask_bias)  [bf16]
  causal: affine_select on diagonal blocks (GPSIMD)
  outT[d,q] = v-blocks.T @ E (bf16), rowsums via ones-matmul broadcast
  y[l,:] += (outT_h * recip_rowsum) @ Wo_h   (bf16)
"""
import sys
if '/opt/trn_rl_repo' not in sys.path:
    sys.path.insert(0, '/opt/trn_rl_repo')

import numpy as np
import ml_dtypes

import concourse.bass as bass
import concourse.mybir as mybir
import concourse.tile as tile
from concourse import bacc
from concourse.bass_utils import run_bass_kernel_spmd

F32 = mybir.dt.float32
F32R = mybir.dt.float32r
BF16 = mybir.dt.bfloat16

B, L, D, H, HKV = 4, 1024, 2048, 16, 4
HD = D // H            # 128
HD2 = HD // 2          # 64
NH = 8                 # heads per core
NKV = 2                # kv heads per core
NB = L // 128          # 8 blocks of 128
AluOp = mybir.AluOpType
Act = mybir.ActivationFunctionType

_CACHED = {}


def _chunks_for_stripe(mb):
    """Q-column chunks [(qs, qe)] covering [128*mb, 1024), split at 256-multiples."""
    q0 = 128 * mb
    out = []
    while q0 < L:
        qe = min(L, (q0 // 256 + 1) * 256)
        out.append((q0, qe))
        q0 = qe
    return out


def build_program():
    nc = bacc.Bacc("TRN2", target_bir_lowering=False, debug=False)

    # ---- dram params (per-core shapes) ----
    xh = nc.declare_dram_parameter("xh", [L, D], BF16, isOutput=False)
    xl = nc.declare_dram_parameter("xl", [L, D], BF16, isOutput=False)
    wq = nc.declare_dram_parameter("wq", [128, 16, NH * 128], F32R, isOutput=False)
    wk = nc.declare_dram_parameter("wk", [128, 16, NKV * 128], F32R, isOutput=False)
    wv = nc.declare_dram_parameter("wv", [128, 16, NKV * 128], F32R, isOutput=False)
    wo = nc.declare_dram_parameter("wo", [128, NH, D], BF16, isOutput=False)
    cosq = nc.declare_dram_parameter("cosq", [128, NH, L], F32, isOutput=False)
    sinq = nc.declare_dram_parameter("sinq", [128, NH, L], F32, isOutput=False)
    cosk = nc.declare_dram_parameter("cosk", [128, NKV, L], F32, isOutput=False)
    sink = nc.declare_dram_parameter("sink", [128, NKV, L], F32, isOutput=False)
    maskb = nc.declare_dram_parameter("maskb", [128, NB], F32, isOutput=False)
    cprime = nc.declare_dram_parameter("cprime", [128, NH], F32, isOutput=False)
    alpha = nc.declare_dram_parameter("alpha", [128, NH], F32, isOutput=False)
    pmrot = nc.declare_dram_parameter("pmrot", [128, 128], F32R, isOutput=False)
    pmswap = nc.declare_dram_parameter("pmswap", [128, 128], F32R, isOutput=False)
    onesb = nc.declare_dram_parameter("onesb", [128, 128], BF16, isOutput=False)
    identb = nc.declare_dram_parameter("identb", [128, 128], BF16, isOutput=False)
    y = nc.declare_dram_parameter("y", [L, D], F32, isOutput=True)

    with tile.TileContext(nc) as tc:
        with (
            tc.tile_pool(name="persist", bufs=1) as pp,
            tc.tile_pool(name="psum", bufs=1, space="PSUM") as psp,
        ):
            # persistent tiles
            consts = {}
            for nm, src, dt_ in [("pmrot", pmrot, F32R), ("pmswap", pmswap, F32R),
                                 ("onesb", onesb, BF16), ("identb", identb, BF16),
                                 ("maskb", maskb, F32), ("cprime", cprime, F32),
                                 ("alpha", alpha, F32)]:
                t = pp.tile(list(src.shape), dt_, tag=nm, name=nm)
                nc.sync.dma_start(t[:], src[:])
                consts[nm] = t

            xt = [pp.tile([128, L], F32R, tag=f"xt{ib}", name=f"xt{ib}")
                  for ib in range(16)]
            krt = [pp.tile([128, L], F32R, tag=f"krt{g}", name=f"krt{g}")
                   for g in range(NKV)]
            kswap = [pp.tile([128, L], F32R, tag=f"ksw{g}", name=f"ksw{g}")
                     for g in range(NKV)]
            vblk = [pp.tile([128, 128], BF16, tag=f"vb{i}", name=f"vb{i}")
                    for i in range(NKV * NB)]
            outtn = [pp.tile([128, L], BF16, tag=f"ot{h}", name=f"ot{h}")
                     for h in range(NH)]

            # ---------------- prologue: xT + k/v proj + k rope + v transpose
            with tc.tile_pool(name="pro", bufs=1) as ppro:
                # x transpose-load + recombine
                for ib in range(16):
                    th = ppro.tile([128, L], BF16, tag="xh_t", bufs=3)
                    tl = ppro.tile([128, L], BF16, tag="xl_t", bufs=3)
                    nc.sync.dma_start_transpose(th[:], xh[:, ib * 128:(ib + 1) * 128])
                    nc.sync.dma_start_transpose(tl[:], xl[:, ib * 128:(ib + 1) * 128])
                    nc.vector.tensor_add(xt[ib][:], th[:], tl[:])

                wk_t = ppro.tile([128, 16, NKV * 128], F32R, tag="wk")
                wv_t = ppro.tile([128, 16, NKV * 128], F32R, tag="wv")
                nc.sync.dma_start(wk_t[:], wk[:])
                nc.sync.dma_start(wv_t[:], wv[:])

                kt_s = []
                for g in range(NKV):
                    ps = psp.tile([128, L], F32, tag="pj", bufs=1)
                    for ib in range(16):
                        for c in range(2):
                            nc.tensor.matmul(
                                ps[:, c * 512:(c + 1) * 512],
                                wk_t[:, ib, g * 128:(g + 1) * 128],
                                xt[ib][:, c * 512:(c + 1) * 512],
                                start=(ib == 0), stop=(ib == 15))
                    kt = ppro.tile([128, L], F32R, tag="kt_s", bufs=2)
                    nc.any.tensor_copy(kt[:], ps[:])
                    kt_s.append(kt)

                # k rope
                for g in range(NKV):
                    psr = psp.tile([128, L], F32, tag="pj", bufs=1)
                    for c in range(2):
                        nc.tensor.matmul(psr[:, c * 512:(c + 1) * 512],
                                         consts["pmrot"][:],
                                         kt_s[g][:, c * 512:(c + 1) * 512])
                    t1 = ppro.tile([128, L], F32, tag="rtmp", bufs=4)
                    t2 = ppro.tile([128, L], F32, tag="rtmp", bufs=4)
                    csl = ppro.tile([128, L], F32, tag="ktab", bufs=4)
                    snl = ppro.tile([128, L], F32, tag="ktab", bufs=4)
                    nc.sync.dma_start(csl[:], cosk[:, g, :])
                    nc.sync.dma_start(snl[:], sink[:, g, :])
                    nc.vector.tensor_mul(t1[:], psr[:], snl[:])
                    nc.vector.tensor_mul(t2[:], kt_s[g][:].bitcast(F32), csl[:])
                    nc.vector.tensor_add(krt[g][:], t1[:], t2[:])
                    # kswap = partition-swap of krt
                    psw = psp.tile([128, L], F32, tag="pj", bufs=1)
                    for c in range(2):
                        nc.tensor.matmul(psw[:, c * 512:(c + 1) * 512],
                                         consts["pmswap"][:],
                                         krt[g][:, c * 512:(c + 1) * 512])
                    nc.any.tensor_copy(kswap[g][:], psw[:])

                # v proj (bf16 out) + transpose to [m, d] blocks
                for g in range(NKV):
                    ps = psp.tile([128, L], F32, tag="pj", bufs=1)
                    for ib in range(16):
                        for c in range(2):
                            nc.tensor.matmul(
                                ps[:, c * 512:(c + 1) * 512],
                                wv_t[:, ib, g * 128:(g + 1) * 128],
                                xt[ib][:, c * 512:(c + 1) * 512],
                                start=(ib == 0), stop=(ib == 15))
                    vt = ppro.tile([128, L], BF16, tag="vt_s", bufs=2)
                    nc.any.tensor_copy(vt[:], ps[:])
                    for mb in range(NB):
                        pv = psp.tile([128, 128], BF16, tag="pj", bufs=1)
                        nc.tensor.transpose(pv[:], vt[:, mb * 128:(mb + 1) * 128],
                                            consts["identb"][:])
                        nc.vector.tensor_copy(vblk[g * NB + mb][:], pv[:])

            # ---------------- head loop
            with tc.tile_pool(name="hl", bufs=1) as ph:
                for h in range(NH):
                    g = h // 4  # local kv head
                    wq_t = ph.tile([128, 16, 128], F32R, tag="wq_h", bufs=2)
                    nc.sync.dma_start(wq_t[:], wq[:, :, h * 128:(h + 1) * 128])
                    cq = ph.tile([128, L], F32, tag="tabq", bufs=2)
                    sq = ph.tile([128, L], F32, tag="tabq", bufs=2)
                    nc.sync.dma_start(cq[:], cosq[:, h, :])
                    nc.sync.dma_start(sq[:], sinq[:, h, :])

                    psq = psp.tile([128, L], F32, tag="pj", bufs=1)
                    for ib in range(16):
                        for c in range(2):
                            nc.tensor.matmul(
                                psq[:, c * 512:(c + 1) * 512],
                                wq_t[:, ib, :],
                                xt[ib][:, c * 512:(c + 1) * 512],
                                start=(ib == 0), stop=(ib == 15))
                    qt_s = ph.tile([128, L], F32R, tag="qt_s", bufs=2)
                    nc.any.tensor_copy(qt_s[:], psq[:])

                    psr = psp.tile([128, L], F32, tag="pj", bufs=1)
                    for c in range(2):
                        nc.tensor.matmul(psr[:, c * 512:(c + 1) * 512],
                                         consts["pmrot"][:],
                                         qt_s[:, c * 512:(c + 1) * 512])
                    t1 = ph.tile([128, L], F32, tag="qtmp", bufs=2)
                    t2 = ph.tile([128, L], F32, tag="qtmp", bufs=2)
                    nc.vector.tensor_mul(t1[:], psr[:], sq[:])
                    nc.vector.tensor_mul(t2[:], qt_s[:].bitcast(F32), cq[:])
                    qrt = ph.tile([128, L], F32R, tag="qrt", bufs=2)
                    nc.vector.tensor_add(qrt[:], t1[:], t2[:])

                    # scores -> E tiles
                    etiles = []
                    for mb in range(NB):
                        w = L - 128 * mb
                        et = ph.tile([128, w], BF16, tag=f"esc{mb}", bufs=3,
                                     name=f"esc_h{mb}")
                        etiles.append(et)
                    for mb in range(NB):
                        kb = slice(mb * 128, (mb + 1) * 128)
                        for (qs, qe) in _chunks_for_stripe(mb):
                            s = qe - qs
                            psA = psp.tile([128, 2 * s], F32, tag="scA", bufs=1,
                                           name="psA")
                            psB = psp.tile([128, 2 * s], F32, tag="scB", bufs=1,
                                           name="psB")
                            nc.tensor.matmul(psA[:, 0:s], krt[g][0:64, kb],
                                             qrt[0:64, qs:qe])
                            nc.tensor.matmul(psA[:, s:2 * s], kswap[g][0:64, kb],
                                             qrt[0:64, qs:qe])
                            nc.tensor.matmul(psB[:, 0:s], krt[g][64:128, kb],
                                             qrt[64:128, qs:qe])
                            nc.tensor.matmul(psB[:, s:2 * s], kswap[g][64:128, kb],
                                             qrt[64:128, qs:qe])
                            bs = ph.tile([128, 2 * s], F32, tag="bs", bufs=3)
                            nc.any.tensor_copy(bs[:], psB[:])
                            tp = ph.tile([128, 2 * s], F32, tag="tprod", bufs=3)
                            nc.vector.tensor_mul(tp[:], psA[:], bs[:])
                            raw = ph.tile([128, s], F32, tag="raw", bufs=3)
                            nc.vector.scalar_tensor_tensor(
                                raw[:], tp[:, s:2 * s], consts["cprime"][:, h:h + 1],
                                tp[:, 0:s], op0=AluOp.mult, op1=AluOp.add)
                            esl = etiles[mb][:, qs - 128 * mb: qe - 128 * mb]
                            nc.scalar.activation(esl, raw[:], Act.Exp,
                                                 bias=consts["maskb"][:, mb:mb + 1],
                                                 scale=consts["alpha"][:, h:h + 1])
                            if qs == 128 * mb:
                                # causal triangle on the diagonal 128 cols
                                nc.gpsimd.affine_select(
                                    etiles[mb][:, 0:128], etiles[mb][:, 0:128],
                                    pattern=[[1, 128]], compare_op=AluOp.is_ge,
                                    fill=0.0, base=0, channel_multiplier=-1)

                    # attnv + rowsum
                    ps_o = psp.tile([128, L], F32, tag="acco", bufs=1, name="ps_o")
                    ps_rs = psp.tile([128, L], F32, tag="accr", bufs=1, name="ps_rs")
                    for c in range(2):
                        mbs = [mb for mb in range(NB) if 128 * mb < 512 * (c + 1)]
                        for i, mb in enumerate(mbs):
                            os_ = max(512 * c, 128 * mb)
                            oe = 512 * (c + 1)
                            esl = etiles[mb][:, os_ - 128 * mb: oe - 128 * mb]
                            st, sp = (i == 0), (i == len(mbs) - 1)
                            nc.tensor.matmul(ps_o[:, os_:oe], vblk[g * NB + mb][:],
                                             esl, start=st, stop=sp)
                            nc.tensor.matmul(ps_rs[:, os_:oe], consts["onesb"][:],
                                             esl, start=st, stop=sp)
                    rcp = ph.tile([128, L], F32, tag="rcp", bufs=1)
                    nc.vector.reciprocal_approx_fast(rcp[:], ps_rs[:])
                    nc.vector.tensor_mul(outtn[h][:], ps_o[:], rcp[:])

            # ---------------- epilogue: Wo projection
            with tc.tile_pool(name="ep", bufs=1) as pe:
                wo_t = []
                for hb in range(NH):
                    t = pe.tile([128, D], BF16, tag=f"wo{hb}", name=f"wo{hb}")
                    nc.sync.dma_start(t[:], wo[:, hb, :])
                    wo_t.append(t)
                for lb in range(NB):
                    for c in range(2):
                        psy = psp.tile([128, 1024], F32, tag="pj", bufs=1, name="psy")
                        for cc in range(2):
                            for hh in range(NH):
                                nc.tensor.matmul(
                                    psy[:, cc * 512:(cc + 1) * 512],
                                    outtn[hh][:, lb * 128:(lb + 1) * 128],
                                    wo_t[hh][:, c * 1024 + cc * 512:
                                            c * 1024 + (cc + 1) * 512],
                                    start=(hh == 0), stop=(hh == NH - 1))
                        yt = pe.tile([128, 1024], F32, tag="ytile", bufs=3)
                        nc.any.tensor_copy(yt[:], psy[:])
                        nc.sync.dma_start(
                            y[lb * 128:(lb + 1) * 128, c * 1024:(c + 1) * 1024], yt[:])

    nc.compile()
    return nc


def _host_prep(x, Wq, Wk, Wv, Wo, q_param, log_scale, cos, sin, mask):
    """Build the 8 per-core input maps."""
    x = np.asarray(x, np.float32)
    Wq = np.asarray(Wq, np.float32)
    Wk = np.asarray(Wk, np.float32)
    Wv = np.asarray(Wv, np.float32)
    Wo = np.asarray(Wo, np.float32)
    cos = np.asarray(cos, np.float32)[0]      # [L, H, 64]
    sin = np.asarray(sin, np.float32)[0]
    qp = np.asarray(q_param, np.float32).reshape(H)
    ls = np.asarray(log_scale, np.float32).reshape(H)
    mask = np.asarray(mask)

    p64 = np.arange(128) % 64

    PM = np.zeros((128, 128), np.float32)
    for dp in range(128):
        base, r = (dp // 64) * 64, dp % 64
        if r < 32:
            PM[base + r + 32, dp] = -1.0
        else:
            PM[base + r - 32, dp] = 1.0
    SW = np.zeros((128, 128), np.float32)
    for dp in range(128):
        SW[(dp + 64) % 128, dp] = 1.0
    ONES = np.ones((128, 128), ml_dtypes.bfloat16)
    IDENT = np.eye(128, dtype=ml_dtypes.bfloat16)

    in_maps = []
    for core in range(8):
        b, g2 = core // 2, core % 2
        heads = list(range(g2 * NH, (g2 + 1) * NH))
        kvs = list(range(g2 * NKV, (g2 + 1) * NKV))

        xb = x[b]
        xh = xb.astype(ml_dtypes.bfloat16)
        xlo = (xb - xh.astype(np.float32)).astype(ml_dtypes.bfloat16)

        wq_c = Wq[:, g2 * NH * 128:(g2 + 1) * NH * 128]
        wk_c = Wk[:, g2 * NKV * 128:(g2 + 1) * NKV * 128]
        wv_c = Wv[:, g2 * NKV * 128:(g2 + 1) * NKV * 128]
        wo_c = Wo[g2 * NH * 128:(g2 + 1) * NH * 128, :]

        wq_p = wq_c.reshape(16, 128, NH * 128).transpose(1, 0, 2).copy()
        wk_p = wk_c.reshape(16, 128, NKV * 128).transpose(1, 0, 2).copy()
        wv_p = wv_c.reshape(16, 128, NKV * 128).transpose(1, 0, 2).copy()
        wo_p = wo_c.reshape(NH, 128, D).transpose(1, 0, 2).astype(ml_dtypes.bfloat16)

        cosq_p = np.ascontiguousarray(cos[:, heads, :][:, :, p64].transpose(2, 1, 0))
        sinq_p = np.ascontiguousarray(sin[:, heads, :][:, :, p64].transpose(2, 1, 0))
        cosk_p = np.ascontiguousarray(cos[:, kvs, :][:, :, p64].transpose(2, 1, 0))
        sink_p = np.ascontiguousarray(sin[:, kvs, :][:, :, p64].transpose(2, 1, 0))

        mb = np.where(mask[b].reshape(NB, 128).T.astype(bool), 0.0, -1e9)
        mb = mb.astype(np.float32)

        cpr = np.tile((-2.0 * np.tanh(qp[heads]))[None, :], (128, 1))
        alp = np.tile((np.exp(ls[heads]) / HD)[None, :], (128, 1))

        in_maps.append({
            "xh": xh, "xl": xlo,
            "wq": wq_p.astype(np.float32), "wk": wk_p.astype(np.float32),
            "wv": wv_p.astype(np.float32), "wo": wo_p,
            "cosq": cosq_p, "sinq": sinq_p, "cosk": cosk_p, "sink": sink_p,
            "maskb": mb, "cprime": cpr.astype(np.float32),
            "alpha": alp.astype(np.float32),
            "pmrot": PM, "pmswap": SW, "onesb": ONES, "identb": IDENT,
        })
    return in_maps


def kernel(**inputs):
    if "nc" not in _CACHED:
        _CACHED["nc"] = build_program()
    nc = _CACHED["nc"]
    in_maps = _host_prep(**inputs)
    res = run_bass_kernel_spmd(nc, in_maps, list(range(8))).results
    out = np.empty((B, L, D), np.float32)
    for b in range(B):
        out[b] = res[2 * b]["y"] + res[2 * b + 1]["y"]
    return out



# revision 7
# speedup vs baseline: 1.3299x; 1.3299x over previous
"""BivectorRotarySelfAttention TRN2 kernel (bf16 pipeline).

Sharding: 8 cores = 4 batches x 2 head-halves. Each core computes one batch's
attention for 8 heads (2 kv heads) and a partial output projection; host sums
the two head-half partials per batch.

Per-core dataflow (transposed layouts: features in partitions, seq in free):
  xT[ib]  = dma_transpose(x_bf16)                       16 x [128, L] bf16
  qT/kT/vT = W.T @ xT   (bf16 matmuls, PSUM f32, copied out as bf16)
  rope: psr = pmrot@qt (PE); t1 = psr*sin (DVE); t2 = qt*cos (Pool);
        qrt = t1+t2 (Pool)
  scores S^T[m,q]: 4 K=64 bf16 matmuls per 256-col chunk:
        psA = [S0 | c'*C0] (krt/kswap_h), psB = [S1 | C1] (krt/kswap)
        bs  = copy(psB) (Act), tp = psA*bs (DVE),
        raw = tp[:s] + tp[s:] (Pool), E = exp(alpha*raw + maskbias) (Act)
  causal: affine_select on diagonal blocks (Pool)
  outT[d,q] = vblk.T @ E; rowsums via ones-matmul; outtn = ps_o * rcp (DVE)
  y[l,:] = sum_h outtn_h.T @ Wo_h  (bf16 matmuls, f32 out)
"""
import sys
if '/opt/trn_rl_repo' not in sys.path:
    sys.path.insert(0, '/opt/trn_rl_repo')

import numpy as np
import ml_dtypes

import concourse.bass as bass
import concourse.mybir as mybir
import concourse.tile as tile
from concourse import bacc
from concourse.bass_utils import run_bass_kernel_spmd

F32 = mybir.dt.float32
BF16 = mybir.dt.bfloat16

B, L, D, H, HKV = 4, 1024, 2048, 16, 4
HD = D // H            # 128
HD2 = HD // 2          # 64
NH = 8                 # heads per core
NKV = 2                # kv heads per core
NB = L // 128          # 8 blocks of 128
AluOp = mybir.AluOpType
Act = mybir.ActivationFunctionType

_CACHED = {}


def _chunks_for_stripe(mb):
    """Q-column chunks [(qs, qe)] covering [128*mb, 1024), split at 256-multiples."""
    q0 = 128 * mb
    out = []
    while q0 < L:
        qe = min(L, (q0 // 256 + 1) * 256)
        out.append((q0, qe))
        q0 = qe
    return out


# packed E-tile column offsets: region for stripe mb starts at _EOFF[mb]
_EOFF = [0]
for _mb in range(NB):
    _EOFF.append(_EOFF[-1] + (L - 128 * _mb))
_ETOT = _EOFF[NB]          # 4608


def build_program():
    nc = bacc.Bacc("TRN2", target_bir_lowering=False, debug=False)

    # ---- dram params (per-core shapes) ----
    xh = nc.declare_dram_parameter("xh", [L, D], BF16, isOutput=False)
    wq = nc.declare_dram_parameter("wq", [128, NH, 16, 128], BF16, isOutput=False)
    wk = nc.declare_dram_parameter("wk", [128, 16, NKV * 128], BF16, isOutput=False)
    wv = nc.declare_dram_parameter("wv", [128, 16, NKV * 128], BF16, isOutput=False)
    wo = nc.declare_dram_parameter("wo", [128, NH, D], BF16, isOutput=False)
    cosq = nc.declare_dram_parameter("cosq", [128, NH, L], BF16, isOutput=False)
    sinq = nc.declare_dram_parameter("sinq", [128, NH, L], BF16, isOutput=False)
    cosk = nc.declare_dram_parameter("cosk", [128, NKV, L], BF16, isOutput=False)
    sink = nc.declare_dram_parameter("sink", [128, NKV, L], BF16, isOutput=False)
    maskb = nc.declare_dram_parameter("maskb", [128, NB], F32, isOutput=False)
    cprime = nc.declare_dram_parameter("cprime", [128, NH], F32, isOutput=False)
    alpha = nc.declare_dram_parameter("alpha", [128, NH], F32, isOutput=False)
    pmrot = nc.declare_dram_parameter("pmrot", [128, 128], BF16, isOutput=False)
    pmswap = nc.declare_dram_parameter("pmswap", [128, 128], BF16, isOutput=False)
    onesb = nc.declare_dram_parameter("onesb", [128, 128], BF16, isOutput=False)
    identb = nc.declare_dram_parameter("identb", [128, 128], BF16, isOutput=False)
    y = nc.declare_dram_parameter("y", [L, D], F32, isOutput=True)

    with tile.TileContext(nc) as tc:
        with (
            tc.tile_pool(name="persist", bufs=1) as pp,
            tc.tile_pool(name="psum", bufs=1, space="PSUM") as psp,
        ):
            # PSUM tags: "qp" [128,1024] bufs=1 (2 banks) for projections/rope,
            # "sc" [128,512] bufs=6 (6 banks) for scores/attnv/vT/epilogue.
            def qp_tile():
                return psp.tile([128, L], F32, tag="qp", bufs=1, name="qp_t")

            def sc_tile(w=512, dt_=F32):
                return psp.tile([128, w], dt_, tag="sc", bufs=6, name="sc_t")

            # persistent tiles / consts
            consts = {}
            for nm, src, dt_ in [("pmrot", pmrot, BF16), ("pmswap", pmswap, BF16),
                                 ("onesb", onesb, BF16), ("identb", identb, BF16),
                                 ("maskb", maskb, F32), ("cprime", cprime, F32),
                                 ("alpha", alpha, F32)]:
                t = pp.tile(list(src.shape), dt_, tag=nm, name=nm)
                nc.sync.dma_start(t[:], src[:])
                consts[nm] = t

            wk_t = pp.tile([128, 16, NKV * 128], BF16, tag="wk", name="wk_t")
            wv_t = pp.tile([128, 16, NKV * 128], BF16, tag="wv", name="wv_t")
            nc.sync.dma_start(wk_t[:], wk[:])
            nc.sync.dma_start(wv_t[:], wv[:])
            csl = pp.tile([128, NKV, L], BF16, tag="cosk", name="csl")
            snl = pp.tile([128, NKV, L], BF16, tag="sink", name="snl")
            nc.sync.dma_start(csl[:], cosk[:])
            nc.sync.dma_start(snl[:], sink[:])

            xt = [pp.tile([128, L], BF16, tag=f"xt{ib}", name=f"xt{ib}")
                  for ib in range(16)]
            for ib in range(16):
                nc.sync.dma_start_transpose(xt[ib][:], xh[:, ib * 128:(ib + 1) * 128])

            krt = [pp.tile([128, L], BF16, tag=f"krt{g}", name=f"krt{g}")
                   for g in range(NKV)]
            kswap = [pp.tile([128, L], BF16, tag=f"ksw{g}", name=f"ksw{g}")
                     for g in range(NKV)]
            vblk = [pp.tile([128, 128], BF16, tag=f"vb{i}", name=f"vb{i}")
                    for i in range(NKV * NB)]
            outtn = [pp.tile([128, L], BF16, tag=f"ot{h}", name=f"ot{h}")
                     for h in range(NH)]

            # ---------------- prologue: k/v proj + k rope + v transpose
            with tc.tile_pool(name="pro", bufs=1) as ppro:
                for g in range(NKV):
                    # k projection
                    psk = qp_tile()
                    for ib in range(16):
                        for c in range(2):
                            nc.tensor.matmul(
                                psk[:, c * 512:(c + 1) * 512],
                                wk_t[:, ib, g * 128:(g + 1) * 128],
                                xt[ib][:, c * 512:(c + 1) * 512],
                                start=(ib == 0), stop=(ib == 15))
                    kt = ppro.tile([128, L], BF16, tag="kt_s", bufs=2, name="kt")
                    nc.scalar.copy(kt[:], psk[:])
                    # k rope
                    psrk = qp_tile()
                    for c in range(2):
                        nc.tensor.matmul(psrk[:, c * 512:(c + 1) * 512],
                                         consts["pmrot"][:],
                                         kt[:, c * 512:(c + 1) * 512])
                    t1k = ppro.tile([128, L], BF16, tag="rtmp", bufs=2, name="t1k")
                    t2k = ppro.tile([128, L], BF16, tag="rtmp", bufs=2, name="t2k")
                    nc.vector.tensor_mul(t1k[:], psrk[:], snl[:, g, :])
                    nc.gpsimd.tensor_mul(t2k[:], kt[:], csl[:, g, :])
                    nc.gpsimd.tensor_add(krt[g][:], t1k[:], t2k[:])
                    # kswap = partition-swap of krt
                    pswk = qp_tile()
                    for c in range(2):
                        nc.tensor.matmul(pswk[:, c * 512:(c + 1) * 512],
                                         consts["pmswap"][:],
                                         krt[g][:, c * 512:(c + 1) * 512])
                    nc.scalar.copy(kswap[g][:], pswk[:])
                    # v projection + transpose to [m, d] blocks
                    psv = qp_tile()
                    for ib in range(16):
                        for c in range(2):
                            nc.tensor.matmul(
                                psv[:, c * 512:(c + 1) * 512],
                                wv_t[:, ib, g * 128:(g + 1) * 128],
                                xt[ib][:, c * 512:(c + 1) * 512],
                                start=(ib == 0), stop=(ib == 15))
                    vt = ppro.tile([128, L], BF16, tag="vt_s", bufs=2, name="vt")
                    nc.scalar.copy(vt[:], psv[:])
                    for mb in range(NB):
                        pv = sc_tile(128, BF16)
                        nc.tensor.transpose(pv[:], vt[:, mb * 128:(mb + 1) * 128],
                                            consts["identb"][:])
                        if mb % 2 == 0:
                            nc.vector.tensor_copy(vblk[g * NB + mb][:], pv[:])
                        else:
                            nc.scalar.copy(vblk[g * NB + mb][:], pv[:])

            # ---------------- head loop
            with tc.tile_pool(name="hl", bufs=1) as ph:
                for h in range(NH):
                    g = h // 4  # local kv head
                    wq_t = ph.tile([128, 16, 128], BF16, tag="wq_h", bufs=2,
                                   name="wq_t")
                    nc.sync.dma_start(wq_t[:], wq[:, h, :, :])
                    cq = ph.tile([128, L], BF16, tag="cq", bufs=2, name="cq")
                    sq = ph.tile([128, L], BF16, tag="sq", bufs=2, name="sq")
                    nc.sync.dma_start(cq[:], cosq[:, h, :])
                    nc.sync.dma_start(sq[:], sinq[:, h, :])

                    # q projection
                    psq = qp_tile()
                    for ib in range(16):
                        for c in range(2):
                            nc.tensor.matmul(
                                psq[:, c * 512:(c + 1) * 512],
                                wq_t[:, ib, :],
                                xt[ib][:, c * 512:(c + 1) * 512],
                                start=(ib == 0), stop=(ib == 15))
                    qt_s = ph.tile([128, L], BF16, tag="qt_s", bufs=2, name="qt_s")
                    nc.scalar.copy(qt_s[:], psq[:])

                    # q rope
                    psr = qp_tile()
                    for c in range(2):
                        nc.tensor.matmul(psr[:, c * 512:(c + 1) * 512],
                                         consts["pmrot"][:],
                                         qt_s[:, c * 512:(c + 1) * 512])
                    t1 = ph.tile([128, L], BF16, tag="qtmp", bufs=2, name="t1")
                    t2 = ph.tile([128, L], BF16, tag="qtmp", bufs=2, name="t2")
                    nc.vector.tensor_mul(t1[:], psr[:], sq[:])
                    nc.gpsimd.tensor_mul(t2[:], qt_s[:], cq[:])
                    qrt = ph.tile([128, L], BF16, tag="qrt", bufs=2, name="qrt")
                    nc.gpsimd.tensor_add(qrt[:], t1[:], t2[:])

                    # per-head scaled kswap: kswap_h = c'(h) * kswap[g]
                    kswap_h = ph.tile([128, L], BF16, tag="ksw_h", bufs=2,
                                      name="kswap_h")
                    nc.vector.tensor_scalar_mul(
                        kswap_h[:], kswap[g][:], consts["cprime"][:, h:h + 1])

                    # scores -> packed E tile [128, 4608]
                    etile = ph.tile([128, _ETOT], BF16, tag="esc", bufs=2,
                                    name="etile")
                    for mb in range(NB):
                        kb = slice(mb * 128, (mb + 1) * 128)
                        w = L - 128 * mb
                        rawt = ph.tile([128, w], BF16, tag="raw", bufs=3,
                                       name="rawt")
                        for ci, (qs, qe) in enumerate(_chunks_for_stripe(mb)):
                            s = qe - qs
                            psA = sc_tile()
                            psB = sc_tile()
                            nc.tensor.matmul(psA[:, 0:s], krt[g][0:64, kb],
                                             qrt[0:64, qs:qe])
                            nc.tensor.matmul(psA[:, s:2 * s], kswap_h[0:64, kb],
                                             qrt[0:64, qs:qe])
                            nc.tensor.matmul(psB[:, 0:s], krt[g][64:128, kb],
                                             qrt[64:128, qs:qe])
                            nc.tensor.matmul(psB[:, s:2 * s], kswap[g][64:128, kb],
                                             qrt[64:128, qs:qe])
                            bs = ph.tile([128, 512], BF16, tag="bs", bufs=3,
                                         name="bs")
                            nc.scalar.copy(bs[:, 0:2 * s], psB[:, 0:2 * s])
                            tp = ph.tile([128, 512], BF16, tag="tprod", bufs=3,
                                         name="tp")
                            nc.vector.tensor_mul(tp[:, 0:2 * s], psA[:, 0:2 * s],
                                                 bs[:, 0:2 * s])
                            nc.gpsimd.tensor_add(
                                rawt[:, qs - 128 * mb:qe - 128 * mb],
                                tp[:, 0:s], tp[:, s:2 * s])
                        esl = etile[:, _EOFF[mb]:_EOFF[mb] + w]
                        nc.scalar.activation(esl, rawt[:], Act.Exp,
                                             bias=consts["maskb"][:, mb:mb + 1],
                                             scale=consts["alpha"][:, h:h + 1])
                        # causal triangle on the diagonal 128 cols
                        nc.gpsimd.affine_select(
                            etile[:, _EOFF[mb]:_EOFF[mb] + 128],
                            etile[:, _EOFF[mb]:_EOFF[mb] + 128],
                            pattern=[[1, 128]], compare_op=AluOp.is_ge,
                            fill=0.0, base=0, channel_multiplier=-1)

                    # attnv + rowsum, per 512-col half
                    for c in range(2):
                        mbs = [mb for mb in range(NB) if 128 * mb < 512 * (c + 1)]
                        ps_o = sc_tile()
                        ps_rs = sc_tile()
                        for i, mb in enumerate(mbs):
                            os_ = max(512 * c, 128 * mb)
                            oe = 512 * (c + 1)
                            esl = etile[:, _EOFF[mb] + os_ - 128 * mb:
                                        _EOFF[mb] + oe - 128 * mb]
                            st, sp = (i == 0), (i == len(mbs) - 1)
                            nc.tensor.matmul(ps_o[:, os_ - 512 * c:oe - 512 * c],
                                             vblk[g * NB + mb][:], esl,
                                             start=st, stop=sp)
                            nc.tensor.matmul(ps_rs[:, os_ - 512 * c:oe - 512 * c],
                                             consts["onesb"][:], esl,
                                             start=st, stop=sp)
                        rcp = ph.tile([128, 512], F32, tag="rcp", bufs=2,
                                      name="rcp")
                        nc.vector.reciprocal_approx_fast(rcp[:], ps_rs[:])
                        nc.vector.tensor_mul(outtn[h][:, c * 512:(c + 1) * 512],
                                             ps_o[:], rcp[:])

            # ---------------- epilogue: Wo projection
            with tc.tile_pool(name="ep", bufs=1) as pe:
                wo_t = []
                for hb in range(NH):
                    t = pe.tile([128, D], BF16, tag=f"wo{hb}", name=f"wo{hb}")
                    nc.sync.dma_start(t[:], wo[:, hb, :])
                    wo_t.append(t)
                for lb in range(NB):
                    for c in range(2):
                        yt = pe.tile([128, 1024], F32, tag="ytile", bufs=4,
                                     name="yt")
                        for cc in range(2):
                            psy = sc_tile()
                            for hh in range(NH):
                                nc.tensor.matmul(
                                    psy[:],
                                    outtn[hh][:, lb * 128:(lb + 1) * 128],
                                    wo_t[hh][:, c * 1024 + cc * 512:
                                             c * 1024 + (cc + 1) * 512],
                                    start=(hh == 0), stop=(hh == NH - 1))
                            if cc == 0:
                                nc.vector.tensor_copy(yt[:, 0:512], psy[:])
                            else:
                                nc.scalar.copy(yt[:, 512:1024], psy[:])
                        nc.sync.dma_start(
                            y[lb * 128:(lb + 1) * 128, c * 1024:(c + 1) * 1024],
                            yt[:])

    nc.compile()
    return nc


def _host_prep(x, Wq, Wk, Wv, Wo, q_param, log_scale, cos, sin, mask):
    """Build the 8 per-core input maps."""
    x = np.asarray(x, np.float32)
    Wq = np.asarray(Wq, np.float32)
    Wk = np.asarray(Wk, np.float32)
    Wv = np.asarray(Wv, np.float32)
    Wo = np.asarray(Wo, np.float32)
    cos = np.asarray(cos, np.float32)[0]      # [L, H, 64]
    sin = np.asarray(sin, np.float32)[0]
    qp = np.asarray(q_param, np.float32).reshape(H)
    ls = np.asarray(log_scale, np.float32).reshape(H)
    mask = np.asarray(mask)

    p64 = np.arange(128) % 64

    PM = np.zeros((128, 128), np.float32)
    for dp in range(128):
        base, r = (dp // 64) * 64, dp % 64
        if r < 32:
            PM[base + r + 32, dp] = -1.0
        else:
            PM[base + r - 32, dp] = 1.0
    SW = np.zeros((128, 128), np.float32)
    for dp in range(128):
        SW[(dp + 64) % 128, dp] = 1.0
    PM = PM.astype(ml_dtypes.bfloat16)
    SW = SW.astype(ml_dtypes.bfloat16)
    ONES = np.ones((128, 128), ml_dtypes.bfloat16)
    IDENT = np.eye(128, dtype=ml_dtypes.bfloat16)

    in_maps = []
    for core in range(8):
        b, g2 = core // 2, core % 2
        heads = list(range(g2 * NH, (g2 + 1) * NH))
        kvs = list(range(g2 * NKV, (g2 + 1) * NKV))

        xh = x[b].astype(ml_dtypes.bfloat16)

        wq_c = Wq[:, g2 * NH * 128:(g2 + 1) * NH * 128]
        wk_c = Wk[:, g2 * NKV * 128:(g2 + 1) * NKV * 128]
        wv_c = Wv[:, g2 * NKV * 128:(g2 + 1) * NKV * 128]
        wo_c = Wo[g2 * NH * 128:(g2 + 1) * NH * 128, :]

        # wq: [128(part=K slice), NH, 16(ib), 128(dq)]
        wq_p = wq_c.reshape(16, 128, NH, 128).transpose(1, 2, 0, 3)
        wq_p = np.ascontiguousarray(wq_p).astype(ml_dtypes.bfloat16)
        wk_p = wk_c.reshape(16, 128, NKV * 128).transpose(1, 0, 2)
        wk_p = np.ascontiguousarray(wk_p).astype(ml_dtypes.bfloat16)
        wv_p = wv_c.reshape(16, 128, NKV * 128).transpose(1, 0, 2)
        wv_p = np.ascontiguousarray(wv_p).astype(ml_dtypes.bfloat16)
        wo_p = wo_c.reshape(NH, 128, D).transpose(1, 0, 2)
        wo_p = np.ascontiguousarray(wo_p).astype(ml_dtypes.bfloat16)

        cosq_p = np.ascontiguousarray(
            cos[:, heads, :][:, :, p64].transpose(2, 1, 0)).astype(ml_dtypes.bfloat16)
        sinq_p = np.ascontiguousarray(
            sin[:, heads, :][:, :, p64].transpose(2, 1, 0)).astype(ml_dtypes.bfloat16)
        cosk_p = np.ascontiguousarray(
            cos[:, kvs, :][:, :, p64].transpose(2, 1, 0)).astype(ml_dtypes.bfloat16)
        sink_p = np.ascontiguousarray(
            sin[:, kvs, :][:, :, p64].transpose(2, 1, 0)).astype(ml_dtypes.bfloat16)

        mb = np.where(mask[b].reshape(NB, 128).T.astype(bool), 0.0, -1e9)
        mb = mb.astype(np.float32)

        cpr = np.tile((-2.0 * np.tanh(qp[heads]))[None, :], (128, 1))
        alp = np.tile((np.exp(ls[heads]) / HD)[None, :], (128, 1))

        in_maps.append({
            "xh": xh,
            "wq": wq_p, "wk": wk_p, "wv": wv_p, "wo": wo_p,
            "cosq": cosq_p, "sinq": sinq_p, "cosk": cosk_p, "sink": sink_p,
            "maskb": mb, "cprime": cpr.astype(np.float32),
            "alpha": alp.astype(np.float32),
            "pmrot": PM, "pmswap": SW, "onesb": ONES, "identb": IDENT,
        })
    return in_maps


def kernel(**inputs):
    if "nc" not in _CACHED:
        _CACHED["nc"] = build_program()
    nc = _CACHED["nc"]
    in_maps = _host_prep(**inputs)
    res = run_bass_kernel_spmd(nc, in_maps, list(range(8))).results
    out = np.empty((B, L, D), np.float32)
    for b in range(B):
        out[b] = res[2 * b]["y"] + res[2 * b + 1]["y"]
    return out


# revision 44
# speedup vs baseline: 1.6194x; 1.2176x over previous
"""BivectorRotarySelfAttention TRN2 kernel (bf16 pipeline).

Sharding: 8 cores = 4 batches x 2 head-halves. Each core computes one batch's
attention for 8 heads (2 kv heads) and a partial output projection; host sums
the two head-half partials per batch (bf16 partials, f32 sum).

Per-core dataflow (transposed layouts: features in partitions, seq in free):
  xT[ib]  = dma_transpose(x_bf16)                       16 x [128, L] bf16
  qT/kT/vT = W.T @ xT   (bf16 matmuls, PSUM f32, copied out as bf16)
  rope (per 512-half): psr = pmrot@qt (PE); t1 = psr*sin (DVE);
        t2 = qt*cos (Pool); qrt = t1+t2 (Pool)
  scores S^T[m,q]: 4 K=64 bf16 matmuls per 256-col chunk:
        psA = [S0 | c'*C0] (krt/kswap_h), psB = [S1 | C1] (krt/kswap)
        bs  = copy(psB) (Act), tp = psA*bs (DVE),
        raw = tp[:s] + tp[s:] (Pool), E = exp(alpha*raw + maskbias) (Act)
  causal: affine_select on diagonal blocks (Pool)
  outT[d,q] = vblk.T @ E; rowsums via ones-matmul; outtn = ps_o * rcp (DVE)
  y[l,:] = sum_h outtn_h.T @ Wo_h  (bf16 matmuls, bf16 out)
"""
import sys
if '/opt/trn_rl_repo' not in sys.path:
    sys.path.insert(0, '/opt/trn_rl_repo')

import numpy as np
import ml_dtypes

import concourse.bass as bass
import concourse.mybir as mybir
import concourse.tile as tile
from concourse import bacc
from concourse.bass_utils import run_bass_kernel_spmd

F32 = mybir.dt.float32
BF16 = mybir.dt.bfloat16

B, L, D, H, HKV = 4, 1024, 2048, 16, 4
HD = D // H            # 128
HD2 = HD // 2          # 64
NH = 8                 # heads per core
NKV = 2                # kv heads per core
NB = L // 128          # 8 blocks of 128
AluOp = mybir.AluOpType
Act = mybir.ActivationFunctionType

_CACHED = {}


def _chunks_for_stripe(mb):
    """Q-column chunks [(qs, qe)] covering [128*mb, 1024), split at 256-multiples."""
    q0 = 128 * mb
    out = []
    while q0 < L:
        qe = min(L, (q0 // 256 + 1) * 256)
        out.append((q0, qe))
        q0 = qe
    return out


# packed E-tile column offsets: region for stripe mb starts at _EOFF[mb]
_EOFF = [0]
for _mb in range(NB):
    _EOFF.append(_EOFF[-1] + (L - 128 * _mb))
_ETOT = _EOFF[NB]          # 4608


def build_program():
    nc = bacc.Bacc("TRN2", target_bir_lowering=False, debug=False)

    # ---- dram params (per-core shapes) ----
    xh = nc.declare_dram_parameter("xh", [L, D], BF16, isOutput=False)
    wq = nc.declare_dram_parameter("wq", [128, NH, 16, 128], BF16, isOutput=False)
    wk = nc.declare_dram_parameter("wk", [128, NKV, 16, 128], BF16, isOutput=False)
    wv = nc.declare_dram_parameter("wv", [128, NKV, 16, 128], BF16, isOutput=False)
    wo = nc.declare_dram_parameter("wo", [128, NH, D], BF16, isOutput=False)
    cosq = nc.declare_dram_parameter("cosq", [128, NH, L], BF16, isOutput=False)
    sinq = nc.declare_dram_parameter("sinq", [128, NH, L], BF16, isOutput=False)
    cosk = nc.declare_dram_parameter("cosk", [128, NKV, L], BF16, isOutput=False)
    sink = nc.declare_dram_parameter("sink", [128, NKV, L], BF16, isOutput=False)
    maskb = nc.declare_dram_parameter("maskb", [128, NB], F32, isOutput=False)
    cprime = nc.declare_dram_parameter("cprime", [128, NH], F32, isOutput=False)
    alpha = nc.declare_dram_parameter("alpha", [128, NH], F32, isOutput=False)
    pmrot = nc.declare_dram_parameter("pmrot", [128, 128], BF16, isOutput=False)
    pmswap = nc.declare_dram_parameter("pmswap", [128, 128], BF16, isOutput=False)
    onesb = nc.declare_dram_parameter("onesb", [128, 128], BF16, isOutput=False)
    identb = nc.declare_dram_parameter("identb", [128, 128], BF16, isOutput=False)
    y = nc.declare_dram_parameter("y", [L, D], BF16, isOutput=True)

    with tile.TileContext(nc) as tc:
        with (
            tc.tile_pool(name="persist", bufs=1) as pp,
            tc.tile_pool(name="psum", bufs=1, space="PSUM") as psp,
        ):
            # PSUM tags: "qp" [128,1024] bufs=1 (2 banks) for q projections,
            # "sc" [128,512] bufs=6 (6 banks) for scores/attnv/vT/epilogue.
            def qp_tile():
                return psp.tile([128, L], F32, tag="qp", bufs=1, name="qp_t")

            def sc_tile(w=512, dt_=F32):
                return psp.tile([128, w], dt_, tag="sc", bufs=6, name="sc_t")

            # --- DMA order: weight copies first, ONE xbar switch, then all
            # 16 transposes back-to-back (copy<->transpose switches cost ~2.2us)
            wk_t = pp.tile([128, NKV, 16, 128], BF16, tag="wk", name="wk_t")
            wv_t = pp.tile([128, NKV, 16, 128], BF16, tag="wv", name="wv_t")
            xt = [pp.tile([128, L], BF16, tag=f"xt{ib}", name=f"xt{ib}")
                  for ib in range(16)]
            nc.sync.dma_start(wk_t[:, 0], wk[:, 0])
            nc.sync.dma_start(wv_t[:, 0], wv[:, 0])
            nc.sync.dma_start(wk_t[:, 1], wk[:, 1])
            nc.sync.dma_start(wv_t[:, 1], wv[:, 1])
            for ib in range(16):
                nc.sync.dma_start_transpose(xt[ib][:], xh[:, ib * 128:(ib + 1) * 128])

            # small consts + k tables + head-0 tables next
            consts = {}
            for nm, src, dt_ in [("pmrot", pmrot, BF16), ("pmswap", pmswap, BF16),
                                 ("onesb", onesb, BF16), ("identb", identb, BF16),
                                 ("maskb", maskb, F32), ("cprime", cprime, F32),
                                 ("alpha", alpha, F32)]:
                t = pp.tile(list(src.shape), dt_, tag=nm, name=nm)
                nc.sync.dma_start(t[:], src[:])
                consts[nm] = t
            csl = pp.tile([128, NKV, L], BF16, tag="cosk", name="csl")
            snl = pp.tile([128, NKV, L], BF16, tag="sink", name="snl")
            nc.sync.dma_start(csl[:], cosk[:])
            nc.sync.dma_start(snl[:], sink[:])

            krt = [pp.tile([128, L], BF16, tag=f"krt{g}", name=f"krt{g}")
                   for g in range(NKV)]
            kswap = [pp.tile([128, L], BF16, tag=f"ksw{g}", name=f"ksw{g}")
                     for g in range(NKV)]
            vblk = [pp.tile([128, 128], BF16, tag=f"vb{i}", name=f"vb{i}")
                    for i in range(NKV * NB)]
            outtn = [pp.tile([128, L], BF16, tag=f"ot{h}", name=f"ot{h}")
                     for h in range(NH)]
            wo_t = [pp.tile([128, D], BF16, tag=f"wo{hb}", name=f"wo{hb}")
                    for hb in range(NH)]

            # ---------------- prologue: k/v proj pipelined via sc psum slots
            with (tc.tile_pool(name="pro", bufs=1) as ppro,
                  tc.tile_pool(name="hl", bufs=1) as ph):
                kt_s, vt_s = [], []
                projs = []
                for g in range(NKV):
                    projs.append((wk_t, g, kt_s, f"kt{g}"))
                    projs.append((wv_t, g, vt_s, f"vt{g}"))
                for w_t, g, outl, tg in projs:
                    pj = [sc_tile(), sc_tile()]
                    for ib in range(16):
                        for c in range(2):
                            nc.tensor.matmul(
                                pj[c][:],
                                w_t[:, g, ib, :],
                                xt[ib][:, c * 512:(c + 1) * 512],
                                start=(ib == 0), stop=(ib == 15))
                    ot = ppro.tile([128, L], BF16, tag=tg, name="projout")
                    if tg.startswith("kt"):
                        nc.scalar.copy(ot[:, 0:512], pj[0][:])
                        nc.scalar.copy(ot[:, 512:1024], pj[1][:])
                    else:
                        nc.vector.tensor_copy(ot[:, 0:512], pj[0][:])
                        nc.vector.tensor_copy(ot[:, 512:1024], pj[1][:])
                    outl.append(ot)

                # v transposes (fill PE while k copies/ropes progress)
                for g in range(NKV):
                    for mb in range(NB):
                        pv = sc_tile(128, BF16)
                        nc.tensor.transpose(pv[:], vt_s[g][:, mb * 128:(mb + 1) * 128],
                                            consts["identb"][:])
                        if mb % 2 == 0:
                            nc.vector.tensor_copy(vblk[g * NB + mb][:], pv[:])
                        else:
                            nc.scalar.copy(vblk[g * NB + mb][:], pv[:])

                # k rotate matmuls
                psrk = {}
                for g in range(NKV):
                    psrk[g] = [sc_tile(), sc_tile()]
                    for c in range(2):
                        nc.tensor.matmul(psrk[g][c][:], consts["pmrot"][:],
                                         kt_s[g][:, c * 512:(c + 1) * 512])
                for g in range(NKV):
                    for c in range(2):
                        cs = slice(c * 512, (c + 1) * 512)
                        t1k = ppro.tile([128, 512], BF16, tag="rtmp", bufs=2,
                                        name="t1k")
                        t2k = ppro.tile([128, 512], BF16, tag="rtmp", bufs=2,
                                        name="t2k")
                        nc.vector.tensor_mul(t1k[:], psrk[g][c][:], snl[:, g, cs])
                        nc.gpsimd.tensor_mul(t2k[:], kt_s[g][:, cs], csl[:, g, cs])
                        nc.gpsimd.tensor_add(krt[g][:, cs], t1k[:], t2k[:])

                # ---------------- head-pipeline helpers
                qs_state = {}

                def q_dma(h):
                    st = {}
                    st["wq"] = ph.tile([128, 16, 128], BF16, tag="wq_h", bufs=2,
                                       name="wq_t")
                    nc.sync.dma_start(st["wq"][:], wq[:, h, :, :])
                    st["cq"] = ph.tile([128, L], BF16, tag="cq", bufs=2, name="cq")
                    st["sq"] = ph.tile([128, L], BF16, tag="sq", bufs=2, name="sq")
                    nc.sync.dma_start(st["cq"][:], cosq[:, h, :])
                    nc.sync.dma_start(st["sq"][:], sinq[:, h, :])
                    qs_state[h] = st

                def q_finish(h):
                    st = qs_state[h]
                    st["qt"] = ph.tile([128, L], BF16, tag="qt_s", bufs=2,
                                       name="qt_s")
                    nc.scalar.copy(st["qt"][:, 0:512], st["psqt"][:, 0:512])
                    nc.vector.tensor_copy(st["qt"][:, 512:1024],
                                          st["psqt"][:, 512:1024])
                    st["ksw_h"] = ph.tile([128, L], BF16, tag="ksw_h", bufs=2,
                                          name="kswap_h")
                    nc.vector.tensor_scalar_mul(
                        st["ksw_h"][:], kswap[h // 4][:],
                        consts["cprime"][:, h:h + 1])

                def q_rope(h, c):
                    st = qs_state[h]
                    if c == 0:
                        st["qrt"] = ph.tile([128, L], BF16, tag="qrt", bufs=2,
                                            name="qrt")
                    cs = slice(c * 512, (c + 1) * 512)
                    psr = sc_tile()
                    nc.tensor.matmul(psr[:], consts["pmrot"][:], st["qt"][:, cs])
                    t1 = ph.tile([128, 512], BF16, tag="qtmp", bufs=4, name="t1")
                    t2 = ph.tile([128, 512], BF16, tag="qtmp", bufs=4, name="t2")
                    nc.vector.tensor_mul(t1[:], psr[:], st["sq"][:, cs])
                    nc.gpsimd.tensor_mul(t2[:], st["qt"][:, cs], st["cq"][:, cs])
                    nc.gpsimd.tensor_add(st["qrt"][:, cs], t1[:], t2[:])

                def attnv_units(h, c):
                    """Closures: accumulation steps + rowsums + normalize."""
                    st = qs_state[h]
                    g = h // 4
                    mbs = [mb for mb in range(NB) if 128 * mb < 512 * (c + 1)]
                    box = {}

                    def mk_step(i, mb):
                        def step():
                            if i == 0:
                                box["ps_o"] = sc_tile()
                            etile = st["etile"]
                            os_ = max(512 * c, 128 * mb)
                            oe = 512 * (c + 1)
                            esl = etile[:, _EOFF[mb] + os_ - 128 * mb:
                                        _EOFF[mb] + oe - 128 * mb]
                            st_, sp = (i == 0), (i == len(mbs) - 1)
                            nc.tensor.matmul(
                                box["ps_o"][:, os_ - 512 * c:oe - 512 * c],
                                vblk[g * NB + mb][:], esl, start=st_, stop=sp)
                        return step

                    def rowsums():
                        etile = st["etile"]
                        ps_rs = sc_tile()
                        box["ps_rs"] = ps_rs
                        for i, mb in enumerate(mbs):
                            os_ = max(512 * c, 128 * mb)
                            oe = 512 * (c + 1)
                            esl = etile[:, _EOFF[mb] + os_ - 128 * mb:
                                        _EOFF[mb] + oe - 128 * mb]
                            nc.tensor.matmul(
                                ps_rs[:, os_ - 512 * c:oe - 512 * c],
                                consts["onesb"][:], esl,
                                start=(i == 0), stop=(i == len(mbs) - 1))

                    def fin():
                        rcp = ph.tile([128, 512], F32, tag="rcp", bufs=2,
                                      name="rcp")
                        nc.vector.reciprocal_approx_fast(rcp[:], box["ps_rs"][:])
                        nc.vector.tensor_mul(
                            outtn[h][:, c * 512:(c + 1) * 512],
                            box["ps_o"][:], rcp[:])

                    return ([mk_step(i, mb) for i, mb in enumerate(mbs)]
                            + [rowsums, fin])

                def attnv_half(h, c):
                    for u in attnv_units(h, c):
                        u()

                def qproj_units(h):
                    def mk(ib):
                        def step():
                            q_proj_ib(h, ib)
                        return step
                    return [mk(ib) for ib in range(16)]

                # ---- epilogue group machinery (also used as head-7 filler)
                egroups = [(lb, c, cc) for lb in range(NB) for c in range(2)
                           for cc in range(2)]
                epi_pre = {}     # group -> held psum tile (hh 0..6 accumulated)
                epi_part = {}    # group -> sbuf bf16 partial (hh 0..6)

                def psy_mm(psy, lb, c, cc, hh, st_, sp):
                    nc.tensor.matmul(
                        psy[:],
                        outtn[hh][:, lb * 128:(lb + 1) * 128],
                        wo_t[hh][:, c * 1024 + cc * 512:
                                 c * 1024 + (cc + 1) * 512],
                        start=st_, stop=sp)

                def epi_pre_units(grp):
                    def mk(hh):
                        def step():
                            if hh == 0:
                                epi_pre[grp] = sc_tile()
                            psy_mm(epi_pre[grp], *grp, hh, hh == 0, False)
                        return step
                    return [mk(hh) for hh in range(NH - 1)]

                def epi_part_units(grp, di):
                    box = {}

                    def mk(hh):
                        def step():
                            if hh == 0:
                                box["psy"] = sc_tile()
                            psy_mm(box["psy"], *grp, hh, hh == 0,
                                   hh == NH - 2)
                        return step

                    def cp():
                        pt = ph.tile([128, 512], BF16, tag="epart", bufs=8,
                                     name="epart")
                        epi_part[grp] = pt
                        if di % 2 == 0:
                            nc.vector.tensor_copy(pt[:], box["psy"][:])
                        else:
                            nc.scalar.copy(pt[:], box["psy"][:])
                    return [mk(hh) for hh in range(NH - 1)] + [cp]

                def q_proj_ib(h, ib):
                    st = qs_state[h]
                    if ib == 0:
                        st["psqt"] = qp_tile()
                    for c in range(2):
                        nc.tensor.matmul(
                            st["psqt"][:, c * 512:(c + 1) * 512],
                            st["wq"][:, ib, :],
                            xt[ib][:, c * 512:(c + 1) * 512],
                            start=(ib == 0), stop=(ib == 15))

                # ---------------- software-pipelined head loop
                q_dma(0)
                q_dma(1)
                for ib in range(16):
                    q_proj_ib(0, ib)
                q_finish(0)
                # kswap = partition-halves swap of krt (pmswap permutation mm)
                pswk = {}
                for g in range(NKV):
                    pswk[g] = [sc_tile(), sc_tile()]
                    for c in range(2):
                        nc.tensor.matmul(pswk[g][c][:], consts["pmswap"][:],
                                         krt[g][:, c * 512:(c + 1) * 512])
                for g in range(NKV):
                    nc.scalar.copy(kswap[g][:, 0:512], pswk[g][0][:])
                    nc.scalar.copy(kswap[g][:, 512:1024], pswk[g][1][:])
                q_rope(0, 0)
                q_rope(0, 1)

                for h in range(NH):
                    st = qs_state[h]
                    g = h // 4
                    if h < NH - 2:
                        q_dma(h + 2)
                    if h == 4:
                        for hb in range(NH):
                            nc.sync.dma_start(wo_t[hb][:], wo[:, hb, :])
                    st["etile"] = ph.tile([128, _ETOT], BF16, tag="esc", bufs=2,
                                          name="etile")
                    etile = st["etile"]
                    qrt = st["qrt"]
                    kswap_h = st["ksw_h"]
                    # PE filler units, popped between score chunks:
                    fillers = []
                    if h > 0:
                        fillers += attnv_units(h - 1, 1)
                    if h < NH - 1:
                        fillers += qproj_units(h + 1)
                        fillers.append(lambda hh=h + 1: q_finish(hh))
                    else:
                        # last head: fill with epilogue pre-accumulation
                        for grp in egroups[:2]:
                            fillers += epi_pre_units(grp)
                        for di, grp in enumerate(egroups[2:10]):
                            fillers += epi_part_units(grp, di)
                    fi = [0]

                    def pop_fill(n):
                        while fi[0] < len(fillers) and n > 0:
                            fillers[fi[0]]()
                            fi[0] += 1
                            n -= 1

                    ci = 0
                    for mb in range(NB):
                        kb = slice(mb * 128, (mb + 1) * 128)
                        w = L - 128 * mb
                        if mb == 6 and h < NH - 1:
                            q_rope(h + 1, 0)
                        if mb == 7 and h < NH - 1:
                            q_rope(h + 1, 1)
                        rawt = ph.tile([128, w], BF16, tag="raw", bufs=2,
                                       name="rawt")
                        for (qs, qe) in _chunks_for_stripe(mb):
                            s = qe - qs
                            psA = sc_tile()
                            psB = sc_tile()
                            nc.tensor.matmul(psA[:, 0:s], krt[g][0:64, kb],
                                             qrt[0:64, qs:qe])
                            nc.tensor.matmul(psA[:, s:2 * s], kswap_h[0:64, kb],
                                             qrt[0:64, qs:qe])
                            nc.tensor.matmul(psB[:, 0:s], krt[g][64:128, kb],
                                             qrt[64:128, qs:qe])
                            nc.tensor.matmul(psB[:, s:2 * s], kswap[g][64:128, kb],
                                             qrt[64:128, qs:qe])
                            bs = ph.tile([128, 512], BF16, tag="bs", bufs=4,
                                         name="bs")
                            nc.scalar.copy(bs[:, 0:2 * s], psB[:, 0:2 * s])
                            tp = ph.tile([128, 512], BF16, tag="tprod", bufs=4,
                                         name="tp")
                            nc.vector.tensor_mul(tp[:, 0:2 * s], psA[:, 0:2 * s],
                                                 bs[:, 0:2 * s])
                            rsl = rawt[:, qs - 128 * mb:qe - 128 * mb]
                            if ci % 4 == 3:
                                # all-bf16 SBUF add runs in DVE 2x mode
                                nc.vector.tensor_add(
                                    rsl, tp[:, 0:s], tp[:, s:2 * s])
                            else:
                                nc.gpsimd.tensor_add(
                                    rsl, tp[:, 0:s], tp[:, s:2 * s])
                            ci += 1
                            if ci >= 2:
                                pop_fill(2)
                        esl = etile[:, _EOFF[mb]:_EOFF[mb] + w]
                        nc.scalar.activation(esl, rawt[:], Act.Exp,
                                             bias=consts["maskb"][:, mb:mb + 1],
                                             scale=consts["alpha"][:, h:h + 1])
                        # causal triangle on the diagonal 128 cols
                        nc.gpsimd.affine_select(
                            etile[:, _EOFF[mb]:_EOFF[mb] + 128],
                            etile[:, _EOFF[mb]:_EOFF[mb] + 128],
                            pattern=[[1, 128]], compare_op=AluOp.is_ge,
                            fill=0.0, base=0, channel_multiplier=-1)
                        if mb == 5:
                            pop_fill(len(fillers))
                            attnv_half(h, 0)
                    pop_fill(len(fillers))

                # ------------ epilogue: Wo projection (finish)
                attnv_half(NH - 1, 1)

                yts = {}
                for grp in egroups:
                    lb, c, cc = grp
                    if (lb, c) not in yts:
                        yts[(lb, c)] = ph.tile([128, 1024], BF16, tag="ytile",
                                               bufs=3, name="yt")
                    yt = yts[(lb, c)]
                    if grp in epi_pre:
                        psy = epi_pre[grp]
                        psy_mm(psy, lb, c, cc, NH - 1, False, True)
                    elif grp in epi_part:
                        psy = sc_tile()
                        psy_mm(psy, lb, c, cc, NH - 1, True, False)
                        nc.tensor.matmul(psy[:], consts["identb"][:],
                                         epi_part[grp][:], start=False,
                                         stop=True)
                    else:
                        psy = sc_tile()
                        for hh in range(NH):
                            psy_mm(psy, lb, c, cc, hh, hh == 0, hh == NH - 1)
                    if cc == 0:
                        nc.vector.tensor_copy(yt[:, 0:512], psy[:])
                    else:
                        nc.scalar.copy(yt[:, 512:1024], psy[:])
                        nc.sync.dma_start(
                            y[lb * 128:(lb + 1) * 128, c * 1024:(c + 1) * 1024],
                            yt[:])

    nc.compile()
    return nc


def _host_prep(x, Wq, Wk, Wv, Wo, q_param, log_scale, cos, sin, mask):
    """Build the 8 per-core input maps."""
    x = np.asarray(x, np.float32)
    Wq = np.asarray(Wq, np.float32)
    Wk = np.asarray(Wk, np.float32)
    Wv = np.asarray(Wv, np.float32)
    Wo = np.asarray(Wo, np.float32)
    cos = np.asarray(cos, np.float32)[0]      # [L, H, 64]
    sin = np.asarray(sin, np.float32)[0]
    qp = np.asarray(q_param, np.float32).reshape(H)
    ls = np.asarray(log_scale, np.float32).reshape(H)
    mask = np.asarray(mask)

    p64 = np.arange(128) % 64

    PM = np.zeros((128, 128), np.float32)
    for dp in range(128):
        base, r = (dp // 64) * 64, dp % 64
        if r < 32:
            PM[base + r + 32, dp] = -1.0
        else:
            PM[base + r - 32, dp] = 1.0
    SW = np.zeros((128, 128), np.float32)
    for dp in range(128):
        SW[(dp + 64) % 128, dp] = 1.0
    PM = PM.astype(ml_dtypes.bfloat16)
    SW = SW.astype(ml_dtypes.bfloat16)
    ONES = np.ones((128, 128), ml_dtypes.bfloat16)
    IDENT = np.eye(128, dtype=ml_dtypes.bfloat16)

    in_maps = []
    for core in range(8):
        b, g2 = core // 2, core % 2
        heads = list(range(g2 * NH, (g2 + 1) * NH))
        kvs = list(range(g2 * NKV, (g2 + 1) * NKV))

        xh = x[b].astype(ml_dtypes.bfloat16)

        wq_c = Wq[:, g2 * NH * 128:(g2 + 1) * NH * 128]
        wk_c = Wk[:, g2 * NKV * 128:(g2 + 1) * NKV * 128]
        wv_c = Wv[:, g2 * NKV * 128:(g2 + 1) * NKV * 128]
        wo_c = Wo[g2 * NH * 128:(g2 + 1) * NH * 128, :]

        # wq: [128(part=K slice), NH, 16(ib), 128(dq)]
        wq_p = wq_c.reshape(16, 128, NH, 128).transpose(1, 2, 0, 3)
        wq_p = np.ascontiguousarray(wq_p).astype(ml_dtypes.bfloat16)
        wk_p = wk_c.reshape(16, 128, NKV, 128).transpose(1, 2, 0, 3)
        wk_p = np.ascontiguousarray(wk_p).astype(ml_dtypes.bfloat16)
        wv_p = wv_c.reshape(16, 128, NKV, 128).transpose(1, 2, 0, 3)
        wv_p = np.ascontiguousarray(wv_p).astype(ml_dtypes.bfloat16)
        wo_p = wo_c.reshape(NH, 128, D).transpose(1, 0, 2)
        wo_p = np.ascontiguousarray(wo_p).astype(ml_dtypes.bfloat16)

        cosq_p = np.ascontiguousarray(
            cos[:, heads, :][:, :, p64].transpose(2, 1, 0)).astype(ml_dtypes.bfloat16)
        sinq_p = np.ascontiguousarray(
            sin[:, heads, :][:, :, p64].transpose(2, 1, 0)).astype(ml_dtypes.bfloat16)
        cosk_p = np.ascontiguousarray(
            cos[:, kvs, :][:, :, p64].transpose(2, 1, 0)).astype(ml_dtypes.bfloat16)
        sink_p = np.ascontiguousarray(
            sin[:, kvs, :][:, :, p64].transpose(2, 1, 0)).astype(ml_dtypes.bfloat16)

        mb = np.where(mask[b].reshape(NB, 128).T.astype(bool), 0.0, -1e9)
        mb = mb.astype(np.float32)

        cpr = np.tile((-2.0 * np.tanh(qp[heads]))[None, :], (128, 1))
        alp = np.tile((np.exp(ls[heads]) / HD)[None, :], (128, 1))

        in_maps.append({
            "xh": xh,
            "wq": wq_p, "wk": wk_p, "wv": wv_p, "wo": wo_p,
            "cosq": cosq_p, "sinq": sinq_p, "cosk": cosk_p, "sink": sink_p,
            "maskb": mb, "cprime": cpr.astype(np.float32),
            "alpha": alp.astype(np.float32),
            "pmrot": PM, "pmswap": SW, "onesb": ONES, "identb": IDENT,
        })
    return in_maps


def kernel(**inputs):
    if "nc" not in _CACHED:
        _CACHED["nc"] = build_program()
    nc = _CACHED["nc"]
    in_maps = _host_prep(**inputs)
    res = run_bass_kernel_spmd(nc, in_maps, list(range(8))).results
    out = np.empty((B, L, D), np.float32)
    for b in range(B):
        out[b] = (res[2 * b]["y"].astype(np.float32)
                  + res[2 * b + 1]["y"].astype(np.float32))
    return out
